# revision 13
# baseline (speedup 1.0000x reference)
"""GATv2 (2-layer, 2-head) + MLP head on 8 Trainium2 NeuronCores.

v2 design:
- Nodes partitioned across cores by id block; edges routed to the dst owner.
- Gather tables (xw per layer) live in HBM as fp16 [50176, 128]; neighbor
  features are fetched with batched GPSIMD dma_gather ucode (0.34ns/descriptor
  vs ~1us/instruction for per-column indirect DMAs). int16 gather indices
  address at most 32768 rows, so the table is split into two 25088-row halves
  (cores 0-3 / 4-7) and each dst batch gathers its A-half and B-half edges
  into disjoint column ranges.
- Layer-1 table is computed fully locally on every core (x is replicated), so
  no layer-1 exchange. Layer-2 shard is AllGathered once (fp16).
- Per-edge math runs on DVE in fp16 via scalar_tensor_tensor (supports the
  4x_2p fast mode); reductions use contiguous fold-trees (tensor_reduce has no
  fast modes); the softmax-weight multiply uses a duplicated-pair operand so
  no broadcast lands on the last axis. Leaky-relu runs on the Scalar engine as
  parametric relu (same activation table set as Exp: no table reloads), and
  the final sigmoid is computed as 1/(1+exp(-z)) to stay in that set.
"""

import os
import numpy as np

import concourse.bass as bass
import concourse.mybir as mybir
import concourse.tile as tile
from concourse import library_config
from concourse.bass_utils import run_bass_kernel_spmd
from concourse.library_overlay import lower_extended_insts
from concourse.masks import make_identity

N, E, IN, HID, H, OUT = 50000, 800000, 128, 64, 2, 1
HC = H * HID                      # 128
NC_CORES = 8
OWN = N // NC_CORES               # 6250 nodes per core
OWNP = 6272                       # padded to 49*128
NB = OWNP // 128                  # 49 batches of 128 dst nodes
NP_TOT = NC_CORES * OWNP          # 50176 table rows
NT = NP_TOT // 128                # 392 table tiles
HALF = (NC_CORES // 2) * OWNP     # 25088 rows per table half
NEG_SLOPE = 0.2
GCH = 8                           # gather chunk: 8 cols = 1024 descriptors
F32 = mybir.dt.float32
FP16 = mybir.dt.float16
I16 = mybir.dt.int16
AF = mybir.ActivationFunctionType
ALU = mybir.AluOpType


# ---------------------------------------------------------------------------
# toolchain workarounds
# ---------------------------------------------------------------------------

def _split_multiwait_drains(nc):
    """This walrus build only allows one sync-wait on a Drain TPB_CTRL, but
    TileContext's tail drain carries one wait per live proc. Move extra waits
    onto EventSemaphore instructions inserted right before the drain."""
    for f in nc.m.functions:
        for b in f.blocks:
            out, changed = [], False
            for ins in b.instructions:
                si = ins.sync_info
                if si is not None and len(si.on_wait) > 1:
                    waits = list(si.on_wait)
                    for w_i, w in enumerate(waits[:-1]):
                        es = mybir.InstEventSemaphore(name=f"{ins.name}-presplit{w_i}")
                        es.engine = ins.engine
                        es.sync_info = mybir.SyncInfo(on_wait=[w], on_update=[])
                        out.append(es)
                    ins.sync_info = mybir.SyncInfo(
                        on_wait=[waits[-1]], on_update=list(si.on_update)
                    )
                    changed = True
                out.append(ins)
            if changed:
                b.instructions = out


def _patch_walrus_dge():
    """Enable walrus DGE lowering for vector-dynamic-offset (indirect) DMAs."""
    from concourse import bass_utils as bu

    if getattr(bu, "_gat_dge_patched", False):
        return
    orig = bu.run_command

    def patched(argv, **kwargs):
        if argv and "walrus_driver" in str(argv[0]) and any(
            "codegen" in str(a) for a in argv
        ):
            if not any("--dge-levels" in str(a) for a in argv):
                argv = list(argv) + ["--dge-levels=vector_dynamic_offsets"]
        return orig(argv, **kwargs)

    bu.run_command = patched
    bu._gat_dge_patched = True


def _install_ntff_hook():
    """Register the NTFF profiling hook missing from the image's antenv stub
    (used only when GAT_KERNEL_TRACE=1)."""
    import sys, types

    if "antenv.axon_hooks" in sys.modules:
        return
    try:
        from trn_agent_boot.trn_boot import _ntff_profile_via_ctypes

        hook = _ntff_profile_via_ctypes("/opt/axon/libaxon_pjrt.so")
    except Exception:
        hook = None
    mod = types.ModuleType("antenv.axon_hooks")
    mod.get_axon_ntff_profile_hook = lambda: hook
    mod.set_axon_ntff_profile_hook = lambda h: None
    sys.modules["antenv.axon_hooks"] = mod
    import antenv

    antenv.axon_hooks = mod
    from concourse import bass_utils as bu

    bu.upload_artifacts = lambda tmpdir: str(tmpdir)


# ---------------------------------------------------------------------------
# host-side graph preprocessing (edge routing + padding schedule)
# ---------------------------------------------------------------------------

def _pack_idx(lst):
    """int16 list -> [128, len/16] ucode index tile (16-partition wrap,
    replicated for the 8 Q7 cores)."""
    L = len(lst) // 16
    t = lst.reshape(L, 16).T.astype(np.int16)      # t[p, s] = lst[s*16+p]
    return np.tile(t, (8, 1))


def _host_prep(x, edge_index):
    src = np.asarray(edge_index[0]).astype(np.int64)
    dst = np.asarray(edge_index[1]).astype(np.int64)
    deg = np.bincount(dst, minlength=N)

    # per-core degree sort: orig node id <-> padded sigma position
    pos = np.empty(N, np.int64)
    sigma_nodes = np.full(NP_TOT, -1, np.int64)
    for c in range(NC_CORES):
        nodes = np.arange(c * OWN, (c + 1) * OWN)
        order = nodes[np.argsort(deg[nodes], kind="stable")]
        p0 = c * OWNP
        sigma_nodes[p0:p0 + OWN] = order
        pos[order] = p0 + np.arange(OWN)

    ps = pos[src]                  # source sigma position (= table row)
    pdst = pos[dst]
    isB = ps >= HALF

    # shared per-batch column counts for the A/B halves
    cntA = np.bincount(pdst[~isB], minlength=NP_TOT)
    cntB = np.bincount(pdst[isB], minlength=NP_TOT)
    batch_of_pos = (np.arange(NP_TOT) % OWNP) // 128
    KA = np.zeros(NB, np.int64)
    KB = np.zeros(NB, np.int64)
    np.maximum.at(KA, batch_of_pos, cntA)
    np.maximum.at(KB, batch_of_pos, cntB)
    K_tot = KA + KB
    offA = np.concatenate([[0], np.cumsum(KA)]).astype(np.int64)
    offB = np.concatenate([[0], np.cumsum(KB)]).astype(np.int64)
    off_tot = np.concatenate([[0], np.cumsum(K_tot)]).astype(np.int64)
    S_A, S_B, S_tot = int(offA[-1]), int(offB[-1]), int(off_tot[-1])

    # rank of each edge within its (dst, half) group
    key = pdst * 2 + isB
    e_order = np.argsort(key, kind="stable")
    key_s = key[e_order]
    starts = np.searchsorted(key_s, key_s)
    k_arr = np.arange(E) - starts
    ps_s = ps[e_order]
    pdst_s = pdst[e_order]
    isB_s = isB[e_order]
    c_arr, r_arr = np.divmod(pdst_s, OWNP)
    b_arr, row_arr = np.divmod(r_arr, 128)

    # flat gather lists (slot i = k*128 + row inside each batch segment)
    listA = np.zeros((NC_CORES, 128 * S_A), np.int64)
    listB = np.zeros((NC_CORES, 128 * S_B), np.int64)
    maskb = np.full((NC_CORES, 128, S_tot), -1e30, np.float32)
    selA = ~isB_s
    pa = 128 * offA[b_arr[selA]] + k_arr[selA] * 128 + row_arr[selA]
    listA[c_arr[selA], pa] = ps_s[selA]
    pb = 128 * offB[b_arr[isB_s]] + k_arr[isB_s] * 128 + row_arr[isB_s]
    listB[c_arr[isB_s], pb] = ps_s[isB_s] - HALF
    colA = off_tot[b_arr[selA]] + k_arr[selA]
    maskb[c_arr[selA], row_arr[selA], colA] = 0.0
    colB = off_tot[b_arr[isB_s]] + KA[b_arr[isB_s]] + k_arr[isB_s]
    maskb[c_arr[isB_s], row_arr[isB_s], colB] = 0.0

    idxA = np.zeros((NC_CORES, 128, 8 * S_A), np.int16)
    idxB = np.zeros((NC_CORES, 128, 8 * S_B), np.int16)
    for c in range(NC_CORES):
        for b in range(NB):
            if KA[b]:
                sl = slice(128 * offA[b], 128 * (offA[b] + KA[b]))
                idxA[c][:, 8 * offA[b]:8 * (offA[b] + KA[b])] = _pack_idx(listA[c, sl])
            if KB[b]:
                sl = slice(128 * offB[b], 128 * (offB[b] + KB[b]))
                idxB[c][:, 8 * offB[b]:8 * (offB[b] + KB[b])] = _pack_idx(listB[c, sl])

    x = np.asarray(x, np.float32)
    x_sig = np.zeros((NP_TOT, IN), np.float32)
    valid = sigma_nodes >= 0
    x_sig[valid] = x[sigma_nodes[valid]]
    xT_sig = np.ascontiguousarray(x_sig.T.astype(np.float16))

    return dict(
        sigma_nodes=sigma_nodes,
        KA=[int(v) for v in KA], KB=[int(v) for v in KB],
        offA=[int(v) for v in offA], offB=[int(v) for v in offB],
        off_tot=[int(v) for v in off_tot],
        S_A=S_A, S_B=S_B, S_tot=S_tot,
        idxA=idxA, idxB=idxB, maskb=maskb, xT_sig=xT_sig,
        has_deg0=bool((deg == 0).any()),
    )


# ---------------------------------------------------------------------------
# bass program
# ---------------------------------------------------------------------------

def _build_program(prep, use_bias):
    KA, KB = prep["KA"], prep["KB"]
    offA, offB, off_tot = prep["offA"], prep["offB"], prep["off_tot"]
    S_A, S_B, S_tot = prep["S_A"], prep["S_B"], prep["S_tot"]
    has_deg0 = prep["has_deg0"]

    nc = bass.Bass("TRN2", target_bir_lowering=False)

    xT_d = nc.dram_tensor("xT", [128, NP_TOT], FP16, kind="ExternalInput")
    xTown_d = nc.dram_tensor("xTown", [128, OWNP], FP16, kind="ExternalInput")
    idxA_d = nc.dram_tensor("idxA", [128, max(8 * S_A, 16)], I16, kind="ExternalInput")
    idxB_d = nc.dram_tensor("idxB", [128, max(8 * S_B, 16)], I16, kind="ExternalInput")
    maskb_d = nc.dram_tensor("maskb", [128, S_tot], F32, kind="ExternalInput")
    W1T_d = nc.dram_tensor("W1T", [128, HC], FP16, kind="ExternalInput")
    W2T_d = nc.dram_tensor("W2T", [128, HC], FP16, kind="ExternalInput")
    b1m_d = nc.dram_tensor("b1m", [128, HC], F32, kind="ExternalInput")
    b2m_d = nc.dram_tensor("b2m", [128, HC], F32, kind="ExternalInput")
    att1m_d = nc.dram_tensor("att1m", [128, HC], FP16, kind="ExternalInput")
    att2m_d = nc.dram_tensor("att2m", [128, HC], FP16, kind="ExternalInput")
    Wp1T_d = nc.dram_tensor("Wp1T", [128, HID], FP16, kind="ExternalInput")
    bp1_d = nc.dram_tensor("bp1c", [HID, 1], F32, kind="ExternalInput")
    Wp2T_d = nc.dram_tensor("Wp2T", [HID, OUT], FP16, kind="ExternalInput")
    nbp2_d = nc.dram_tensor("nbp2c", [OUT, 1], F32, kind="ExternalInput")

    out_d = nc.dram_tensor("out", [1, OWNP], F32, kind="ExternalOutput")

    tab1 = nc.dram_tensor("tab1", [NP_TOT, HC], FP16)
    tab2 = nc.dram_tensor("tab2", [NP_TOT, HC], FP16)
    xw2own = nc.dram_tensor("xw2own", [OWNP, HC], FP16)

    with tile.TileContext(nc) as tc:
        with (
            tc.tile_pool(name="const", bufs=1) as cpool,
            tc.tile_pool(name="mm", bufs=3) as mmpool,
            tc.tile_pool(name="psum", bufs=2, space="PSUM") as pspool,
            tc.tile_pool(name="gat", bufs=2) as gpool,
            tc.tile_pool(name="w1", bufs=2) as w1pool,
            tc.tile_pool(name="w2", bufs=2) as w2pool,
            tc.tile_pool(name="small", bufs=3) as spool,
        ):
            nc.gpsimd.load_library(library_config.mlp)
            cnt_cache = {}

            def cnt_for(v):
                if v not in cnt_cache:
                    cnt_cache[v] = nc.gpsimd.to_reg(v)
                return cnt_cache[v]

            W1T_sb = cpool.tile([128, HC], FP16)
            W2T_sb = cpool.tile([128, HC], FP16)
            b1m_sb = cpool.tile([128, HC], F32)
            b2m_sb = cpool.tile([128, HC], F32)
            att1m_sb = cpool.tile([128, HC], FP16)
            att2m_sb = cpool.tile([128, HC], FP16)
            Wp1T_sb = cpool.tile([128, HID], FP16)
            bp1_sb = cpool.tile([HID, 1], F32)
            Wp2T_sb = cpool.tile([HID, OUT], FP16)
            nbp2_sb = cpool.tile([OUT, 1], F32)
            ident_sb = cpool.tile([128, 128], F32)
            idxA_sb = cpool.tile([128, max(8 * S_A, 16)], I16)
            idxB_sb = cpool.tile([128, max(8 * S_B, 16)], I16)
            mb_sb = cpool.tile([128, S_tot], F32)
            xi1_sb = cpool.tile([128, NB * HC], FP16)   # resident own xw (layer 1)
            xi2_sb = cpool.tile([128, NB * HC], FP16)   # resident own xw (layer 2)

            for t_sb, t_d in [
                (W1T_sb, W1T_d), (W2T_sb, W2T_d), (b1m_sb, b1m_d),
                (b2m_sb, b2m_d), (att1m_sb, att1m_d), (att2m_sb, att2m_d),
                (Wp1T_sb, Wp1T_d), (bp1_sb, bp1_d), (Wp2T_sb, Wp2T_d),
                (nbp2_sb, nbp2_d), (idxA_sb, idxA_d), (idxB_sb, idxB_d),
                (mb_sb, maskb_d),
            ]:
                nc.sync.dma_start(out=t_sb[:], in_=t_d[:])
            make_identity(nc, ident_sb[:])

            def xw_convert(dst_ap, ps_ap, bias_sb):
                """PSUM f32 -> fp16 (+ optional bias) on the Scalar engine."""
                if use_bias:
                    nc.vector.scalar_tensor_tensor(
                        out=dst_ap, in0=ps_ap, scalar=1.0, in1=bias_sb[:],
                        op0=ALU.bypass, op1=ALU.add)
                else:
                    nc.scalar.activation(out=dst_ap, in_=ps_ap, func=AF.Identity)

            # ---- phase A: own-shard xw1 (SBUF-resident) + full tab1 ----
            for b in range(NB):
                lhsT = mmpool.tile([128, 128], FP16, tag="xwlhs")
                nc.sync.dma_start(out=lhsT[:], in_=xTown_d[:, b * 128:(b + 1) * 128])
                ps = pspool.tile([128, HC], F32, tag="psmm")
                nc.tensor.matmul(out=ps[:], lhsT=lhsT[:], rhs=W1T_sb[:],
                                 start=True, stop=True)
                xw_convert(xi1_sb[:, b * HC:(b + 1) * HC], ps[:], b1m_sb)
            for t in range(NT):
                lhsT = mmpool.tile([128, 128], FP16, tag="xwlhs")
                nc.sync.dma_start(out=lhsT[:], in_=xT_d[:, t * 128:(t + 1) * 128])
                ps = pspool.tile([128, HC], F32, tag="psmm")
                nc.tensor.matmul(out=ps[:], lhsT=lhsT[:], rhs=W1T_sb[:],
                                 start=True, stop=True)
                xw_t = mmpool.tile([128, HC], FP16, tag="xwout")
                xw_convert(xw_t[:], ps[:], b1m_sb)
                nc.scalar.dma_start(out=tab1[t * 128:(t + 1) * 128, :], in_=xw_t[:])

            # ---- GAT layer ----
            def gat_layer(tab, attm_sb, xi_sb, tail):
                for b in range(NB):
                    ka, kb = KA[b], KB[b]
                    K = ka + kb
                    if K == 0:
                        continue
                    # gathers: ucode caps one dma_gather at 1024 descriptors
                    # (16KB SBUF carveout / 16B desc) -> chunk at 8 columns
                    xj = gpool.tile([128, K * HC], FP16, tag="xj")
                    for col0, kk, tslice, isb, ioff in (
                        [(c0, min(GCH, ka - c0), tab[0:HALF, :], idxA_sb,
                          offA[b] + c0) for c0 in range(0, ka, GCH)]
                        + [(ka + c0, min(GCH, kb - c0), tab[HALF:NP_TOT, :],
                            idxB_sb, offB[b] + c0) for c0 in range(0, kb, GCH)]
                    ):
                        nc.gpsimd.dma_gather(
                            out_ap=xj[:, col0 * HC:(col0 + kk) * HC].rearrange(
                                "p (k c) -> p k c", k=kk),
                            in_ap=tslice,
                            idxs_ap=isb[:, 8 * ioff:8 * (ioff + kk)],
                            num_idxs=128 * kk, num_idxs_reg=cnt_for(128 * kk),
                            elem_size=HC)

                    xi_ap = xi_sb[:, b * HC:(b + 1) * HC]
                    xi_b = (xi_ap.rearrange("p (o c) -> p o c", o=1)
                            .broadcast_to([128, K, HC]))
                    # s = xj + xi  (4x)
                    s_t = w1pool.tile([128, K * HC], FP16, tag="work1")
                    nc.vector.scalar_tensor_tensor(
                        out=s_t[:].rearrange("p (k c) -> p k c", k=K),
                        in0=xj[:].rearrange("p (k c) -> p k c", k=K),
                        scalar=1.0, in1=xi_b, op0=ALU.bypass, op1=ALU.add)
                    # e = leaky_relu(s)  (scalar engine, parametric relu)
                    e_t = w2pool.tile([128, K * HC], FP16, tag="work2")
                    nc.scalar.activation(out=e_t[:], in_=s_t[:], func=AF.Prelu,
                                         alpha=NEG_SLOPE)
                    # ea = e * att  (4x), in place over s_t
                    att_b = (attm_sb[:].rearrange("p (o c) -> p o c", o=1)
                             .broadcast_to([128, K, HC]))
                    nc.vector.scalar_tensor_tensor(
                        out=s_t[:].rearrange("p (k c) -> p k c", k=K),
                        in0=e_t[:].rearrange("p (k c) -> p k c", k=K),
                        scalar=1.0, in1=att_b, op0=ALU.bypass, op1=ALU.mult)
                    # fold over c: alpha lands in column 0 of each 64-block
                    ea_v = s_t[:].rearrange("p (kh c) -> p kh c", c=HID)
                    w = HID
                    while w > 1:
                        h2 = w // 2
                        nc.vector.scalar_tensor_tensor(
                            out=ea_v[:, :, 0:h2], in0=ea_v[:, :, 0:h2],
                            scalar=1.0, in1=ea_v[:, :, h2:w],
                            op0=ALU.bypass, op1=ALU.add)
                        w = h2
                    # al = alpha + mask  (compact f32 [128, K*H])
                    al_t = spool.tile([128, K * H], F32, tag="al")
                    mb_t = mb_sb[:, off_tot[b]:off_tot[b] + K]
                    mb_v = (mb_t.rearrange("p (k o q) -> p k o q", o=1, q=1)
                            .broadcast_to([128, K, H, 1]))
                    nc.vector.tensor_tensor(
                        out=al_t[:].rearrange("p (k h q) -> p k h q", h=H, q=1),
                        in0=s_t[:].rearrange("p (k h c) -> p k h c",
                                             h=H, c=HID)[:, :, :, 0:1],
                        in1=mb_v, op=ALU.add)
                    # segment softmax over k
                    m_t = spool.tile([128, H], F32, tag="m")
                    nc.vector.tensor_reduce(
                        out=m_t[:],
                        in_=al_t[:].rearrange("p (k h) -> p h k", h=H),
                        axis=mybir.AxisListType.X, op=ALU.max)
                    m_b = (m_t[:].rearrange("p (o h) -> p o h", o=1)
                           .broadcast_to([128, K, H]))
                    nc.vector.tensor_tensor(
                        out=al_t[:].rearrange("p (k h) -> p k h", h=H),
                        in0=al_t[:].rearrange("p (k h) -> p k h", h=H),
                        in1=m_b, op=ALU.subtract)
                    ex_t = spool.tile([128, K * H], F32, tag="ex")
                    nc.scalar.activation(out=ex_t[:], in_=al_t[:], func=AF.Exp)
                    if has_deg0:
                        m01_t = spool.tile([128, K], F32, tag="m01")
                        nc.vector.tensor_scalar(out=m01_t[:], in0=mb_t,
                                                scalar1=-1.0, scalar2=None,
                                                op0=ALU.is_ge)
                        m01_b = (m01_t[:].rearrange("p (k o) -> p k o", o=1)
                                 .broadcast_to([128, K, H]))
                        nc.vector.tensor_tensor(
                            out=ex_t[:].rearrange("p (k h) -> p k h", h=H),
                            in0=ex_t[:].rearrange("p (k h) -> p k h", h=H),
                            in1=m01_b, op=ALU.mult)
                    s_sum = spool.tile([128, H], F32, tag="ssum")
                    nc.vector.tensor_reduce(
                        out=s_sum[:],
                        in_=ex_t[:].rearrange("p (k h) -> p h k", h=H),
                        axis=mybir.AxisListType.X, op=ALU.add)
                    nc.vector.tensor_scalar_add(out=s_sum[:], in0=s_sum[:],
                                                scalar1=1e-16)
                    rs_t = spool.tile([128, H], F32, tag="rs")
                    nc.vector.reciprocal(out=rs_t[:], in_=s_sum[:])
                    # duplicated-pair fp16 multipliers
                    exd_t = spool.tile([128, K * H * 2], FP16, tag="exd")
                    nc.vector.tensor_scalar(
                        out=exd_t[:].rearrange("p (kh d) -> p kh d", d=2),
                        in0=(ex_t[:].rearrange("p (kh o) -> p kh o", o=1)
                             .broadcast_to([128, K * H, 2])),
                        scalar1=1.0, scalar2=None, op0=ALU.mult)
                    rsd_t = spool.tile([128, H * 2], FP16, tag="rsd")
                    nc.vector.tensor_scalar(
                        out=rsd_t[:].rearrange("p (h d) -> p h d", d=2),
                        in0=(rs_t[:].rearrange("p (h o) -> p h o", o=1)
                             .broadcast_to([128, H, 2])),
                        scalar1=1.0, scalar2=None, op0=ALU.mult)
                    # msg = xj * ex  (2x, paired-duplicate keeps last dim packed)
                    exd_b = (exd_t[:].rearrange("p (kh o d) -> p kh o d",
                                                o=1, d=2)
                             .broadcast_to([128, K * H, HID // 2, 2]))
                    nc.vector.tensor_tensor(
                        out=e_t[:].rearrange("p (kh c d) -> p kh c d",
                                             c=HID // 2, d=2),
                        in0=xj[:].rearrange("p (kh c d) -> p kh c d",
                                            c=HID // 2, d=2),
                        in1=exd_b, op=ALU.mult)
                    # fold over k -> ob in e_t[:, 0:HC]
                    kc = K
                    while kc > 1:
                        if kc % 2 == 1:
                            nc.vector.scalar_tensor_tensor(
                                out=e_t[:, 0:HC], in0=e_t[:, 0:HC], scalar=1.0,
                                in1=e_t[:, (kc - 1) * HC:kc * HC],
                                op0=ALU.bypass, op1=ALU.add)
                            kc -= 1
                            if kc == 1:
                                break
                        h2 = kc // 2
                        nc.vector.scalar_tensor_tensor(
                            out=e_t[:, 0:h2 * HC], in0=e_t[:, 0:h2 * HC],
                            scalar=1.0, in1=e_t[:, h2 * HC:kc * HC],
                            op0=ALU.bypass, op1=ALU.add)
                        kc = h2
                    # normalize (f32 out for the transpose path)
                    rsd_b = (rsd_t[:].rearrange("p (h o d) -> p h o d", o=1, d=2)
                             .broadcast_to([128, H, HID // 2, 2]))
                    ob_t = spool.tile([128, HC], F32, tag="ob")
                    nc.vector.tensor_tensor(
                        out=ob_t[:].rearrange("p (h c d) -> p h c d",
                                              h=H, c=HID // 2, d=2),
                        in0=e_t[:, 0:HC].rearrange("p (h c d) -> p h c d",
                                                   h=H, c=HID // 2, d=2),
                        in1=rsd_b, op=ALU.mult)
                    # transpose + relu -> hT fp16 [feat, nodes]
                    ps_tr = pspool.tile([128, 128], F32, tag="pstr")
                    nc.tensor.transpose(out=ps_tr[:], in_=ob_t[:],
                                        identity=ident_sb[:])
                    hT_t = spool.tile([128, 128], FP16, tag="houtT")
                    nc.scalar.activation(out=hT_t[:], in_=ps_tr[:], func=AF.Relu)
                    tail(b, hT_t)

            # ---- layer 1 (tail computes the layer-2 xw shard) ----
            def tail_l1(b, hT_t):
                ps2 = pspool.tile([128, HC], F32, tag="psmm")
                nc.tensor.matmul(out=ps2[:], lhsT=hT_t[:], rhs=W2T_sb[:],
                                 start=True, stop=True)
                xw_convert(xi2_sb[:, b * HC:(b + 1) * HC], ps2[:], b2m_sb)
                nc.scalar.dma_start(out=xw2own[b * 128:(b + 1) * 128, :],
                                    in_=xi2_sb[:, b * HC:(b + 1) * HC])

            gat_layer(tab1, att1m_sb, xi1_sb, tail_l1)

            nc.gpsimd.collective_compute(
                "AllGather", ALU.bypass,
                replica_groups=[list(range(NC_CORES))],
                ins=[xw2own[:]], outs=[tab2[:]],
            )

            # ---- layer 2 with fused MLP head ----
            def tail_l2(b, hT_t):
                sl = slice(b * 128, (b + 1) * 128)
                ps_z = pspool.tile([HID, 128], F32, tag="psz")
                nc.tensor.matmul(out=ps_z[:], lhsT=Wp1T_sb[:], rhs=hT_t[:],
                                 start=True, stop=True)
                zT = mmpool.tile([HID, 128], FP16, tag="zT")
                nc.scalar.activation(out=zT[:], in_=ps_z[:], func=AF.Identity,
                                     bias=bp1_sb[:])
                ps_o = pspool.tile([OUT, 128], F32, tag="pso")
                nc.tensor.matmul(out=ps_o[:], lhsT=Wp2T_sb[:], rhs=zT[:],
                                 start=True, stop=True)
                # sigmoid(z + bp2) = 1 / (1 + exp(-z - bp2))
                sg_t = spool.tile([OUT, 128], F32, tag="osig")
                nc.scalar.activation(out=sg_t[:], in_=ps_o[:], func=AF.Exp,
                                     scale=-1.0, bias=nbp2_sb[:])
                nc.vector.tensor_scalar_add(out=sg_t[:], in0=sg_t[:], scalar1=1.0)
                o_t = spool.tile([OUT, 128], F32, tag="orecip")
                nc.vector.reciprocal(out=o_t[:], in_=sg_t[:])
                nc.sync.dma_start(out=out_d[:, sl], in_=o_t[:])

            gat_layer(tab2, att2m_sb, xi2_sb, tail_l2)

    _split_multiwait_drains(nc)
    lower_extended_insts(nc)
    return nc


# ---------------------------------------------------------------------------
# entry point
# ---------------------------------------------------------------------------

def kernel(x, edge_index, W1, b1, att1, W2, b2, att2, Wp1, bp1, Wp2, bp2):
    _patch_walrus_dge()
    trace = os.environ.get("GAT_KERNEL_TRACE") == "1"
    if trace:
        _install_ntff_hook()

    prep = _host_prep(x, edge_index)

    W1 = np.asarray(W1, np.float32)
    W2 = np.asarray(W2, np.float32)
    b1 = np.asarray(b1, np.float32)
    b2 = np.asarray(b2, np.float32)
    att1 = np.asarray(att1, np.float32)
    att2 = np.asarray(att2, np.float32)
    Wp1 = np.asarray(Wp1, np.float32)
    bp1 = np.asarray(bp1, np.float32)
    Wp2 = np.asarray(Wp2, np.float32)
    bp2 = np.asarray(bp2, np.float32)

    use_bias = bool(np.any(b1) or np.any(b2))
    nc = _build_program(prep, use_bias)

    W1T = np.ascontiguousarray(W1.T.astype(np.float16))
    W2T = np.ascontiguousarray(W2.T.astype(np.float16))
    b1m = np.broadcast_to(b1[None, :], (128, HC)).astype(np.float32).copy()
    b2m = np.broadcast_to(b2[None, :], (128, HC)).astype(np.float32).copy()
    att1m = np.broadcast_to(att1.reshape(1, HC), (128, HC)).astype(np.float16).copy()
    att2m = np.broadcast_to(att2.reshape(1, HC), (128, HC)).astype(np.float16).copy()
    Wp1T = np.ascontiguousarray(Wp1.T.astype(np.float16))
    Wp2T = np.ascontiguousarray(Wp2.T.astype(np.float16))
    bp1c = bp1.reshape(HID, 1).astype(np.float32).copy()
    nbp2c = (-bp2).reshape(OUT, 1).astype(np.float32).copy()

    xT_sig = prep["xT_sig"]
    idxA_w = max(8 * prep["S_A"], 16)
    idxB_w = max(8 * prep["S_B"], 16)
    in_maps = []
    for c in range(NC_CORES):
        idxA = np.zeros((128, idxA_w), np.int16)
        idxA[:, :8 * prep["S_A"]] = prep["idxA"][c]
        idxB = np.zeros((128, idxB_w), np.int16)
        idxB[:, :8 * prep["S_B"]] = prep["idxB"][c]
        in_maps.append({
            "xT": xT_sig,
            "xTown": np.ascontiguousarray(
                xT_sig[:, c * OWNP:(c + 1) * OWNP]),
            "idxA": idxA, "idxB": idxB,
            "maskb": prep["maskb"][c],
            "W1T": W1T, "W2T": W2T, "b1m": b1m, "b2m": b2m,
            "att1m": att1m, "att2m": att2m,
            "Wp1T": Wp1T, "bp1c": bp1c, "Wp2T": Wp2T, "nbp2c": nbp2c,
        })

    res = run_bass_kernel_spmd(
        nc, in_maps, core_ids=list(range(NC_CORES)), trace=trace,
    )
    if trace:
        print(f"HW exec time: {res.exec_time_ns} ns")

    out = np.zeros((N, OUT), np.float32)
    sigma_nodes = prep["sigma_nodes"]
    for c in range(NC_CORES):
        vals = res.results[c]["out"][0]
        nodes = sigma_nodes[c * OWNP:(c + 1) * OWNP]
        v = nodes >= 0
        out[nodes[v], 0] = vals[v]
    return out


# revision 14
# speedup vs baseline: 1.6941x; 1.6941x over previous
"""GATv2 (2-layer, 2-head) + MLP head on 8 Trainium2 NeuronCores.

v2 design:
- Nodes partitioned across cores by id block; edges routed to the dst owner.
- Gather tables (xw per layer) live in HBM as fp16 [50176, 128]; neighbor
  features are fetched with batched GPSIMD dma_gather ucode (0.34ns/descriptor
  vs ~1us/instruction for per-column indirect DMAs). int16 gather indices
  address at most 32768 rows, so the table is split into two 25088-row halves
  (cores 0-3 / 4-7) and each dst batch gathers its A-half and B-half edges
  into disjoint column ranges.
- Layer-1 table is computed fully locally on every core (x is replicated), so
  no layer-1 exchange. Layer-2 shard is AllGathered once (fp16).
- Per-edge math runs on DVE in fp16 via scalar_tensor_tensor (supports the
  4x_2p fast mode); reductions use contiguous fold-trees (tensor_reduce has no
  fast modes); the softmax-weight multiply uses a duplicated-pair operand so
  no broadcast lands on the last axis. Leaky-relu runs on the Scalar engine as
  parametric relu (same activation table set as Exp: no table reloads), and
  the final sigmoid is computed as 1/(1+exp(-z)) to stay in that set.
"""

import os
import numpy as np

import concourse.bass as bass
import concourse.mybir as mybir
import concourse.tile as tile
from concourse import library_config
from concourse.bass_utils import run_bass_kernel_spmd
from concourse.library_overlay import lower_extended_insts
from concourse.masks import make_identity

N, E, IN, HID, H, OUT = 50000, 800000, 128, 64, 2, 1
HC = H * HID                      # 128
NC_CORES = 8
OWN = N // NC_CORES               # 6250 nodes per core
OWNP = 6272                       # padded to 49*128
NB = OWNP // 128                  # 49 batches of 128 dst nodes
NP_TOT = NC_CORES * OWNP          # 50176 table rows
NT = NP_TOT // 128                # 392 table tiles
HALF = (NC_CORES // 2) * OWNP     # 25088 rows per table half
NEG_SLOPE = 0.2
GCH = 8                           # gather chunk: 8 cols = 1024 descriptors
F32 = mybir.dt.float32
FP16 = mybir.dt.float16
I16 = mybir.dt.int16
AF = mybir.ActivationFunctionType
ALU = mybir.AluOpType


# ---------------------------------------------------------------------------
# toolchain workarounds
# ---------------------------------------------------------------------------

def _split_multiwait_drains(nc):
    """This walrus build only allows one sync-wait on a Drain TPB_CTRL, but
    TileContext's tail drain carries one wait per live proc. Move extra waits
    onto EventSemaphore instructions inserted right before the drain."""
    for f in nc.m.functions:
        for b in f.blocks:
            out, changed = [], False
            for ins in b.instructions:
                si = ins.sync_info
                if si is not None and len(si.on_wait) > 1:
                    waits = list(si.on_wait)
                    for w_i, w in enumerate(waits[:-1]):
                        es = mybir.InstEventSemaphore(name=f"{ins.name}-presplit{w_i}")
                        es.engine = ins.engine
                        es.sync_info = mybir.SyncInfo(on_wait=[w], on_update=[])
                        out.append(es)
                    ins.sync_info = mybir.SyncInfo(
                        on_wait=[waits[-1]], on_update=list(si.on_update)
                    )
                    changed = True
                out.append(ins)
            if changed:
                b.instructions = out


def _patch_walrus_dge():
    """Enable walrus DGE lowering for vector-dynamic-offset (indirect) DMAs."""
    from concourse import bass_utils as bu

    if getattr(bu, "_gat_dge_patched", False):
        return
    orig = bu.run_command

    def patched(argv, **kwargs):
        if argv and "walrus_driver" in str(argv[0]) and any(
            "codegen" in str(a) for a in argv
        ):
            if not any("--dge-levels" in str(a) for a in argv):
                argv = list(argv) + ["--dge-levels=vector_dynamic_offsets"]
        return orig(argv, **kwargs)

    bu.run_command = patched
    bu._gat_dge_patched = True


def _install_ntff_hook():
    """Register the NTFF profiling hook missing from the image's antenv stub
    (used only when GAT_KERNEL_TRACE=1)."""
    import sys, types

    if "antenv.axon_hooks" in sys.modules:
        return
    try:
        from trn_agent_boot.trn_boot import _ntff_profile_via_ctypes

        hook = _ntff_profile_via_ctypes("/opt/axon/libaxon_pjrt.so")
    except Exception:
        hook = None
    mod = types.ModuleType("antenv.axon_hooks")
    mod.get_axon_ntff_profile_hook = lambda: hook
    mod.set_axon_ntff_profile_hook = lambda h: None
    sys.modules["antenv.axon_hooks"] = mod
    import antenv

    antenv.axon_hooks = mod
    from concourse import bass_utils as bu

    bu.upload_artifacts = lambda tmpdir: str(tmpdir)


# ---------------------------------------------------------------------------
# host-side graph preprocessing (edge routing + padding schedule)
# ---------------------------------------------------------------------------

def _pack_idx(lst):
    """int16 list -> [128, len/16] ucode index tile (16-partition wrap,
    replicated for the 8 Q7 cores)."""
    L = len(lst) // 16
    t = lst.reshape(L, 16).T.astype(np.int16)      # t[p, s] = lst[s*16+p]
    return np.tile(t, (8, 1))


def _host_prep(x, edge_index):
    src = np.asarray(edge_index[0]).astype(np.int64)
    dst = np.asarray(edge_index[1]).astype(np.int64)
    deg = np.bincount(dst, minlength=N)

    # per-core degree sort: orig node id <-> padded sigma position
    pos = np.empty(N, np.int64)
    sigma_nodes = np.full(NP_TOT, -1, np.int64)
    for c in range(NC_CORES):
        nodes = np.arange(c * OWN, (c + 1) * OWN)
        order = nodes[np.argsort(deg[nodes], kind="stable")]
        p0 = c * OWNP
        sigma_nodes[p0:p0 + OWN] = order
        pos[order] = p0 + np.arange(OWN)

    ps = pos[src]                  # source sigma position (= table row)
    pdst = pos[dst]
    isB = ps >= HALF

    # shared per-batch column counts for the A/B halves
    cntA = np.bincount(pdst[~isB], minlength=NP_TOT)
    cntB = np.bincount(pdst[isB], minlength=NP_TOT)
    batch_of_pos = (np.arange(NP_TOT) % OWNP) // 128
    KA = np.zeros(NB, np.int64)
    KB = np.zeros(NB, np.int64)
    np.maximum.at(KA, batch_of_pos, cntA)
    np.maximum.at(KB, batch_of_pos, cntB)
    K_tot = KA + KB
    offA = np.concatenate([[0], np.cumsum(KA)]).astype(np.int64)
    offB = np.concatenate([[0], np.cumsum(KB)]).astype(np.int64)
    off_tot = np.concatenate([[0], np.cumsum(K_tot)]).astype(np.int64)
    S_A, S_B, S_tot = int(offA[-1]), int(offB[-1]), int(off_tot[-1])

    # rank of each edge within its (dst, half) group
    key = pdst * 2 + isB
    e_order = np.argsort(key, kind="stable")
    key_s = key[e_order]
    starts = np.searchsorted(key_s, key_s)
    k_arr = np.arange(E) - starts
    ps_s = ps[e_order]
    pdst_s = pdst[e_order]
    isB_s = isB[e_order]
    c_arr, r_arr = np.divmod(pdst_s, OWNP)
    b_arr, row_arr = np.divmod(r_arr, 128)

    # flat gather lists (slot i = k*128 + row inside each batch segment)
    listA = np.zeros((NC_CORES, 128 * S_A), np.int64)
    listB = np.zeros((NC_CORES, 128 * S_B), np.int64)
    maskb = np.full((NC_CORES, 128, S_tot), -1e30, np.float32)
    selA = ~isB_s
    pa = 128 * offA[b_arr[selA]] + k_arr[selA] * 128 + row_arr[selA]
    listA[c_arr[selA], pa] = ps_s[selA]
    pb = 128 * offB[b_arr[isB_s]] + k_arr[isB_s] * 128 + row_arr[isB_s]
    listB[c_arr[isB_s], pb] = ps_s[isB_s] - HALF
    colA = off_tot[b_arr[selA]] + k_arr[selA]
    maskb[c_arr[selA], row_arr[selA], colA] = 0.0
    colB = off_tot[b_arr[isB_s]] + KA[b_arr[isB_s]] + k_arr[isB_s]
    maskb[c_arr[isB_s], row_arr[isB_s], colB] = 0.0

    idxA = np.zeros((NC_CORES, 128, 8 * S_A), np.int16)
    idxB = np.zeros((NC_CORES, 128, 8 * S_B), np.int16)
    for c in range(NC_CORES):
        for b in range(NB):
            if KA[b]:
                sl = slice(128 * offA[b], 128 * (offA[b] + KA[b]))
                idxA[c][:, 8 * offA[b]:8 * (offA[b] + KA[b])] = _pack_idx(listA[c, sl])
            if KB[b]:
                sl = slice(128 * offB[b], 128 * (offB[b] + KB[b]))
                idxB[c][:, 8 * offB[b]:8 * (offB[b] + KB[b])] = _pack_idx(listB[c, sl])

    x = np.asarray(x, np.float32)
    x_sig = np.zeros((NP_TOT, IN), np.float32)
    valid = sigma_nodes >= 0
    x_sig[valid] = x[sigma_nodes[valid]]
    xT_sig = np.ascontiguousarray(x_sig.T.astype(np.float16))

    return dict(
        sigma_nodes=sigma_nodes,
        KA=[int(v) for v in KA], KB=[int(v) for v in KB],
        offA=[int(v) for v in offA], offB=[int(v) for v in offB],
        off_tot=[int(v) for v in off_tot],
        S_A=S_A, S_B=S_B, S_tot=S_tot,
        idxA=idxA, idxB=idxB, maskb=maskb, xT_sig=xT_sig,
        has_deg0=bool((deg == 0).any()),
    )


# ---------------------------------------------------------------------------
# bass program
# ---------------------------------------------------------------------------

def _build_program(prep, use_bias):
    KA, KB = prep["KA"], prep["KB"]
    offA, offB, off_tot = prep["offA"], prep["offB"], prep["off_tot"]
    S_A, S_B, S_tot = prep["S_A"], prep["S_B"], prep["S_tot"]
    has_deg0 = prep["has_deg0"]

    nc = bass.Bass("TRN2", target_bir_lowering=False, num_swdge_queues=4)

    xT_d = nc.dram_tensor("xT", [128, NP_TOT], FP16, kind="ExternalInput")
    xTown_d = nc.dram_tensor("xTown", [128, OWNP], FP16, kind="ExternalInput")
    idxA_d = nc.dram_tensor("idxA", [128, max(8 * S_A, 16)], I16, kind="ExternalInput")
    idxB_d = nc.dram_tensor("idxB", [128, max(8 * S_B, 16)], I16, kind="ExternalInput")
    maskb_d = nc.dram_tensor("maskb", [128, S_tot], F32, kind="ExternalInput")
    W1T_d = nc.dram_tensor("W1T", [128, HC], FP16, kind="ExternalInput")
    W2T_d = nc.dram_tensor("W2T", [128, HC], FP16, kind="ExternalInput")
    b1m_d = nc.dram_tensor("b1m", [128, HC], F32, kind="ExternalInput")
    b2m_d = nc.dram_tensor("b2m", [128, HC], F32, kind="ExternalInput")
    att1m_d = nc.dram_tensor("att1m", [128, HC], FP16, kind="ExternalInput")
    att2m_d = nc.dram_tensor("att2m", [128, HC], FP16, kind="ExternalInput")
    Wp1T_d = nc.dram_tensor("Wp1T", [128, HID], FP16, kind="ExternalInput")
    bp1_d = nc.dram_tensor("bp1c", [HID, 1], F32, kind="ExternalInput")
    Wp2T_d = nc.dram_tensor("Wp2T", [HID, OUT], FP16, kind="ExternalInput")
    nbp2_d = nc.dram_tensor("nbp2c", [OUT, 1], F32, kind="ExternalInput")

    out_d = nc.dram_tensor("out", [1, OWNP], F32, kind="ExternalOutput")

    tab1 = nc.dram_tensor("tab1", [NP_TOT, HC], FP16)
    tab2 = nc.dram_tensor("tab2", [NP_TOT, HC], FP16)
    xw2own = nc.dram_tensor("xw2own", [OWNP, HC], FP16)

    with tile.TileContext(nc) as tc:
        with (
            tc.tile_pool(name="const", bufs=1) as cpool,
            tc.tile_pool(name="mm", bufs=3) as mmpool,
            tc.tile_pool(name="psum", bufs=2, space="PSUM") as pspool,
            tc.tile_pool(name="gat", bufs=2) as gpool,
            tc.tile_pool(name="w1", bufs=2) as w1pool,
            tc.tile_pool(name="w2", bufs=2) as w2pool,
            tc.tile_pool(name="small", bufs=3) as spool,
        ):
            nc.gpsimd.load_library(library_config.mlp)
            cnt_cache = {}

            def cnt_for(v):
                if v not in cnt_cache:
                    cnt_cache[v] = nc.gpsimd.to_reg(v)
                return cnt_cache[v]

            W1T_sb = cpool.tile([128, HC], FP16)
            W2T_sb = cpool.tile([128, HC], FP16)
            b1m_sb = cpool.tile([128, HC], F32)
            b2m_sb = cpool.tile([128, HC], F32)
            att1m_sb = cpool.tile([128, HC], FP16)
            att2m_sb = cpool.tile([128, HC], FP16)
            Wp1T_sb = cpool.tile([128, HID], FP16)
            bp1_sb = cpool.tile([HID, 1], F32)
            Wp2T_sb = cpool.tile([HID, OUT], FP16)
            nbp2_sb = cpool.tile([OUT, 1], F32)
            ident_sb = cpool.tile([128, 128], F32)
            idxA_sb = cpool.tile([128, max(8 * S_A, 16)], I16)
            idxB_sb = cpool.tile([128, max(8 * S_B, 16)], I16)
            mb_sb = cpool.tile([128, S_tot], F32)
            xi1_sb = cpool.tile([128, NB * HC], FP16)   # resident own xw (layer 1)
            xi2_sb = cpool.tile([128, NB * HC], FP16)   # resident own xw (layer 2)

            for t_sb, t_d in [
                (W1T_sb, W1T_d), (W2T_sb, W2T_d), (b1m_sb, b1m_d),
                (b2m_sb, b2m_d), (att1m_sb, att1m_d), (att2m_sb, att2m_d),
                (Wp1T_sb, Wp1T_d), (bp1_sb, bp1_d), (Wp2T_sb, Wp2T_d),
                (nbp2_sb, nbp2_d), (idxA_sb, idxA_d), (idxB_sb, idxB_d),
                (mb_sb, maskb_d),
            ]:
                nc.sync.dma_start(out=t_sb[:], in_=t_d[:])
            make_identity(nc, ident_sb[:])

            def xw_convert(dst_ap, ps_ap, bias_sb):
                """PSUM f32 -> fp16 (+ optional bias) on the Scalar engine."""
                if use_bias:
                    nc.vector.tensor_tensor(
                        out=dst_ap, in0=ps_ap, in1=bias_sb[:], op=ALU.add)
                else:
                    nc.scalar.activation(out=dst_ap, in_=ps_ap, func=AF.Identity)

            # ---- phase A: own-shard xw1 (SBUF-resident) + full tab1 ----
            for b in range(NB):
                lhsT = mmpool.tile([128, 128], FP16, tag="xwlhs")
                nc.sync.dma_start(out=lhsT[:], in_=xTown_d[:, b * 128:(b + 1) * 128])
                ps = pspool.tile([128, HC], F32, tag="psmm")
                nc.tensor.matmul(out=ps[:], lhsT=lhsT[:], rhs=W1T_sb[:],
                                 start=True, stop=True)
                xw_convert(xi1_sb[:, b * HC:(b + 1) * HC], ps[:], b1m_sb)
            for t in range(NT):
                lhsT = mmpool.tile([128, 128], FP16, tag="xwlhs")
                nc.sync.dma_start(out=lhsT[:], in_=xT_d[:, t * 128:(t + 1) * 128])
                ps = pspool.tile([128, HC], F32, tag="psmm")
                nc.tensor.matmul(out=ps[:], lhsT=lhsT[:], rhs=W1T_sb[:],
                                 start=True, stop=True)
                xw_t = mmpool.tile([128, HC], FP16, tag="xwout")
                xw_convert(xw_t[:], ps[:], b1m_sb)
                nc.scalar.dma_start(out=tab1[t * 128:(t + 1) * 128, :], in_=xw_t[:])

            # ---- GAT layer ----
            def gat_layer(tab, attm_sb, xi_sb, tail):
                qn = [0]
                for b in range(NB):
                    ka, kb = KA[b], KB[b]
                    K = ka + kb
                    if K == 0:
                        continue
                    # gathers: ucode caps one dma_gather at 1024 descriptors
                    # (16KB SBUF carveout / 16B desc) -> chunk at 8 columns
                    xj = gpool.tile([128, K * HC], FP16, tag="xj")
                    for col0, kk, tslice, isb, ioff in (
                        [(c0, min(GCH, ka - c0), tab[0:HALF, :], idxA_sb,
                          offA[b] + c0) for c0 in range(0, ka, GCH)]
                        + [(ka + c0, min(GCH, kb - c0), tab[HALF:NP_TOT, :],
                            idxB_sb, offB[b] + c0) for c0 in range(0, kb, GCH)]
                    ):
                        nc.gpsimd.dma_gather(
                            out_ap=xj[:, col0 * HC:(col0 + kk) * HC].rearrange(
                                "p (k c) -> p k c", k=kk),
                            in_ap=tslice,
                            idxs_ap=isb[:, 8 * ioff:8 * (ioff + kk)],
                            num_idxs=128 * kk, num_idxs_reg=cnt_for(128 * kk),
                            elem_size=HC, queue_num=qn[0] % 4)
                        qn[0] += 1

                    xi_ap = xi_sb[:, b * HC:(b + 1) * HC]
                    xi_b = (xi_ap.rearrange("p (o c) -> p o c", o=1)
                            .broadcast_to([128, K, HC]))
                    # s = xj + xi  (4x)
                    s_t = w1pool.tile([128, K * HC], FP16, tag="work1")
                    nc.vector.tensor_tensor(
                        out=s_t[:].rearrange("p (k c) -> p k c", k=K),
                        in0=xj[:].rearrange("p (k c) -> p k c", k=K),
                        in1=xi_b, op=ALU.add)
                    # e = leaky_relu(s)  (scalar engine, parametric relu)
                    e_t = w2pool.tile([128, K * HC], FP16, tag="work2")
                    nc.scalar.activation(out=e_t[:], in_=s_t[:], func=AF.Prelu,
                                         alpha=NEG_SLOPE)
                    # ea = e * att  (4x), in place over s_t
                    att_b = (attm_sb[:].rearrange("p (o c) -> p o c", o=1)
                             .broadcast_to([128, K, HC]))
                    nc.vector.tensor_tensor(
                        out=s_t[:].rearrange("p (k c) -> p k c", k=K),
                        in0=e_t[:].rearrange("p (k c) -> p k c", k=K),
                        in1=att_b, op=ALU.mult)
                    # fold over c: alpha lands in column 0 of each 64-block
                    ea_v = s_t[:].rearrange("p (kh c) -> p kh c", c=HID)
                    w = HID
                    while w > 1:
                        h2 = w // 2
                        nc.vector.tensor_tensor(
                            out=ea_v[:, :, 0:h2], in0=ea_v[:, :, 0:h2],
                            in1=ea_v[:, :, h2:w], op=ALU.add)
                        w = h2
                    # al = alpha + mask  (compact f32 [128, K*H])
                    al_t = spool.tile([128, K * H], F32, tag="al")
                    mb_t = mb_sb[:, off_tot[b]:off_tot[b] + K]
                    mb_v = (mb_t.rearrange("p (k o q) -> p k o q", o=1, q=1)
                            .broadcast_to([128, K, H, 1]))
                    nc.vector.tensor_tensor(
                        out=al_t[:].rearrange("p (k h q) -> p k h q", h=H, q=1),
                        in0=s_t[:].rearrange("p (k h c) -> p k h c",
                                             h=H, c=HID)[:, :, :, 0:1],
                        in1=mb_v, op=ALU.add)
                    # segment softmax over k
                    m_t = spool.tile([128, H], F32, tag="m")
                    nc.vector.tensor_reduce(
                        out=m_t[:],
                        in_=al_t[:].rearrange("p (k h) -> p h k", h=H),
                        axis=mybir.AxisListType.X, op=ALU.max)
                    m_b = (m_t[:].rearrange("p (o h) -> p o h", o=1)
                           .broadcast_to([128, K, H]))
                    nc.vector.tensor_tensor(
                        out=al_t[:].rearrange("p (k h) -> p k h", h=H),
                        in0=al_t[:].rearrange("p (k h) -> p k h", h=H),
                        in1=m_b, op=ALU.subtract)
                    ex_t = spool.tile([128, K * H], F32, tag="ex")
                    nc.scalar.activation(out=ex_t[:], in_=al_t[:], func=AF.Exp)
                    if has_deg0:
                        m01_t = spool.tile([128, K], F32, tag="m01")
                        nc.vector.tensor_scalar(out=m01_t[:], in0=mb_t,
                                                scalar1=-1.0, scalar2=None,
                                                op0=ALU.is_ge)
                        m01_b = (m01_t[:].rearrange("p (k o) -> p k o", o=1)
                                 .broadcast_to([128, K, H]))
                        nc.vector.tensor_tensor(
                            out=ex_t[:].rearrange("p (k h) -> p k h", h=H),
                            in0=ex_t[:].rearrange("p (k h) -> p k h", h=H),
                            in1=m01_b, op=ALU.mult)
                    s_sum = spool.tile([128, H], F32, tag="ssum")
                    nc.vector.tensor_reduce(
                        out=s_sum[:],
                        in_=ex_t[:].rearrange("p (k h) -> p h k", h=H),
                        axis=mybir.AxisListType.X, op=ALU.add)
                    nc.vector.tensor_scalar_add(out=s_sum[:], in0=s_sum[:],
                                                scalar1=1e-16)
                    rs_t = spool.tile([128, H], F32, tag="rs")
                    nc.vector.reciprocal(out=rs_t[:], in_=s_sum[:])
                    # duplicated-pair fp16 multipliers
                    exd_t = spool.tile([128, K * H * 2], FP16, tag="exd")
                    nc.vector.tensor_scalar(
                        out=exd_t[:].rearrange("p (kh d) -> p kh d", d=2),
                        in0=(ex_t[:].rearrange("p (kh o) -> p kh o", o=1)
                             .broadcast_to([128, K * H, 2])),
                        scalar1=1.0, scalar2=None, op0=ALU.mult)
                    rsd_t = spool.tile([128, H * 2], FP16, tag="rsd")
                    nc.vector.tensor_scalar(
                        out=rsd_t[:].rearrange("p (h d) -> p h d", d=2),
                        in0=(rs_t[:].rearrange("p (h o) -> p h o", o=1)
                             .broadcast_to([128, H, 2])),
                        scalar1=1.0, scalar2=None, op0=ALU.mult)
                    # msg = xj * ex  (2x, paired-duplicate keeps last dim packed)
                    exd_b = (exd_t[:].rearrange("p (kh o d) -> p kh o d",
                                                o=1, d=2)
                             .broadcast_to([128, K * H, HID // 2, 2]))
                    nc.vector.tensor_tensor(
                        out=e_t[:].rearrange("p (kh c d) -> p kh c d",
                                             c=HID // 2, d=2),
                        in0=xj[:].rearrange("p (kh c d) -> p kh c d",
                                            c=HID // 2, d=2),
                        in1=exd_b, op=ALU.mult)
                    # fold over k -> ob in e_t[:, 0:HC]
                    kc = K
                    while kc > 1:
                        if kc % 2 == 1:
                            nc.vector.tensor_tensor(
                                out=e_t[:, 0:HC], in0=e_t[:, 0:HC],
                                in1=e_t[:, (kc - 1) * HC:kc * HC], op=ALU.add)
                            kc -= 1
                            if kc == 1:
                                break
                        h2 = kc // 2
                        nc.vector.tensor_tensor(
                            out=e_t[:, 0:h2 * HC], in0=e_t[:, 0:h2 * HC],
                            in1=e_t[:, h2 * HC:kc * HC], op=ALU.add)
                        kc = h2
                    # normalize (f32 out for the transpose path)
                    rsd_b = (rsd_t[:].rearrange("p (h o d) -> p h o d", o=1, d=2)
                             .broadcast_to([128, H, HID // 2, 2]))
                    ob_t = spool.tile([128, HC], F32, tag="ob")
                    nc.vector.tensor_tensor(
                        out=ob_t[:].rearrange("p (h c d) -> p h c d",
                                              h=H, c=HID // 2, d=2),
                        in0=e_t[:, 0:HC].rearrange("p (h c d) -> p h c d",
                                                   h=H, c=HID // 2, d=2),
                        in1=rsd_b, op=ALU.mult)
                    # transpose + relu -> hT fp16 [feat, nodes]
                    ps_tr = pspool.tile([128, 128], F32, tag="pstr")
                    nc.tensor.transpose(out=ps_tr[:], in_=ob_t[:],
                                        identity=ident_sb[:])
                    hT_t = spool.tile([128, 128], FP16, tag="houtT")
                    nc.scalar.activation(out=hT_t[:], in_=ps_tr[:], func=AF.Relu)
                    tail(b, hT_t)

            # ---- layer 1 (tail computes the layer-2 xw shard) ----
            def tail_l1(b, hT_t):
                ps2 = pspool.tile([128, HC], F32, tag="psmm")
                nc.tensor.matmul(out=ps2[:], lhsT=hT_t[:], rhs=W2T_sb[:],
                                 start=True, stop=True)
                xw_convert(xi2_sb[:, b * HC:(b + 1) * HC], ps2[:], b2m_sb)
                nc.scalar.dma_start(out=xw2own[b * 128:(b + 1) * 128, :],
                                    in_=xi2_sb[:, b * HC:(b + 1) * HC])

            gat_layer(tab1, att1m_sb, xi1_sb, tail_l1)

            nc.gpsimd.collective_compute(
                "AllGather", ALU.bypass,
                replica_groups=[list(range(NC_CORES))],
                ins=[xw2own[:]], outs=[tab2[:]],
            )

            # ---- layer 2 with fused MLP head ----
            def tail_l2(b, hT_t):
                sl = slice(b * 128, (b + 1) * 128)
                ps_z = pspool.tile([HID, 128], F32, tag="psz")
                nc.tensor.matmul(out=ps_z[:], lhsT=Wp1T_sb[:], rhs=hT_t[:],
                                 start=True, stop=True)
                zT = mmpool.tile([HID, 128], FP16, tag="zT")
                nc.scalar.activation(out=zT[:], in_=ps_z[:], func=AF.Identity,
                                     bias=bp1_sb[:])
                ps_o = pspool.tile([OUT, 128], F32, tag="pso")
                nc.tensor.matmul(out=ps_o[:], lhsT=Wp2T_sb[:], rhs=zT[:],
                                 start=True, stop=True)
                # sigmoid(z + bp2) = 1 / (1 + exp(-z - bp2))
                sg_t = spool.tile([OUT, 128], F32, tag="osig")
                nc.scalar.activation(out=sg_t[:], in_=ps_o[:], func=AF.Exp,
                                     scale=-1.0, bias=nbp2_sb[:])
                nc.vector.tensor_scalar_add(out=sg_t[:], in0=sg_t[:], scalar1=1.0)
                o_t = spool.tile([OUT, 128], F32, tag="orecip")
                nc.vector.reciprocal(out=o_t[:], in_=sg_t[:])
                nc.sync.dma_start(out=out_d[:, sl], in_=o_t[:])

            gat_layer(tab2, att2m_sb, xi2_sb, tail_l2)

    _split_multiwait_drains(nc)
    lower_extended_insts(nc)
    return nc


# ---------------------------------------------------------------------------
# entry point
# ---------------------------------------------------------------------------

def kernel(x, edge_index, W1, b1, att1, W2, b2, att2, Wp1, bp1, Wp2, bp2):
    _patch_walrus_dge()
    trace = os.environ.get("GAT_KERNEL_TRACE") == "1"
    if trace:
        _install_ntff_hook()

    prep = _host_prep(x, edge_index)

    W1 = np.asarray(W1, np.float32)
    W2 = np.asarray(W2, np.float32)
    b1 = np.asarray(b1, np.float32)
    b2 = np.asarray(b2, np.float32)
    att1 = np.asarray(att1, np.float32)
    att2 = np.asarray(att2, np.float32)
    Wp1 = np.asarray(Wp1, np.float32)
    bp1 = np.asarray(bp1, np.float32)
    Wp2 = np.asarray(Wp2, np.float32)
    bp2 = np.asarray(bp2, np.float32)

    use_bias = bool(np.any(b1) or np.any(b2))
    nc = _build_program(prep, use_bias)

    W1T = np.ascontiguousarray(W1.T.astype(np.float16))
    W2T = np.ascontiguousarray(W2.T.astype(np.float16))
    b1m = np.broadcast_to(b1[None, :], (128, HC)).astype(np.float32).copy()
    b2m = np.broadcast_to(b2[None, :], (128, HC)).astype(np.float32).copy()
    att1m = np.broadcast_to(att1.reshape(1, HC), (128, HC)).astype(np.float16).copy()
    att2m = np.broadcast_to(att2.reshape(1, HC), (128, HC)).astype(np.float16).copy()
    Wp1T = np.ascontiguousarray(Wp1.T.astype(np.float16))
    Wp2T = np.ascontiguousarray(Wp2.T.astype(np.float16))
    bp1c = bp1.reshape(HID, 1).astype(np.float32).copy()
    nbp2c = (-bp2).reshape(OUT, 1).astype(np.float32).copy()

    xT_sig = prep["xT_sig"]
    idxA_w = max(8 * prep["S_A"], 16)
    idxB_w = max(8 * prep["S_B"], 16)
    in_maps = []
    for c in range(NC_CORES):
        idxA = np.zeros((128, idxA_w), np.int16)
        idxA[:, :8 * prep["S_A"]] = prep["idxA"][c]
        idxB = np.zeros((128, idxB_w), np.int16)
        idxB[:, :8 * prep["S_B"]] = prep["idxB"][c]
        in_maps.append({
            "xT": xT_sig,
            "xTown": np.ascontiguousarray(
                xT_sig[:, c * OWNP:(c + 1) * OWNP]),
            "idxA": idxA, "idxB": idxB,
            "maskb": prep["maskb"][c],
            "W1T": W1T, "W2T": W2T, "b1m": b1m, "b2m": b2m,
            "att1m": att1m, "att2m": att2m,
            "Wp1T": Wp1T, "bp1c": bp1c, "Wp2T": Wp2T, "nbp2c": nbp2c,
        })

    res = run_bass_kernel_spmd(
        nc, in_maps, core_ids=list(range(NC_CORES)), trace=trace,
    )
    if trace:
        print(f"HW exec time: {res.exec_time_ns} ns")

    out = np.zeros((N, OUT), np.float32)
    sigma_nodes = prep["sigma_nodes"]
    for c in range(NC_CORES):
        vals = res.results[c]["out"][0]
        nodes = sigma_nodes[c * OWNP:(c + 1) * OWNP]
        v = nodes >= 0
        out[nodes[v], 0] = vals[v]
    return out
